# revision 1
# baseline (speedup 1.0000x reference)
"""Fused MoE (top-2 routing) on 8 trn2 NeuronCores, expert-parallel.

Strategy: E=16 experts are sharded 2-per-core. The host groups the T*TOPK
(token, slot) pairs by expert (the all-to-all "dispatch"), pads each expert's
token list to a fixed capacity CAP, and ships each core pre-transposed,
SBUF-layout-matched blocks:
  - xT  [2*128, 8*CAP]   gathered tokens: row el*128+p, col kc*CAP+j holds
                         x[token j of expert el, k=kc*128+p]
  - wup [2*128, 8*512]   up_weight[e].T in the same [p, kc, f] layout
  - wdn [4*128, 1024]    down_weight[e].T, row (el*2+hh)*128+p, col kout
  - wv  [128, 10]        routing weight per pair, [p, chunk] layout
Every DMA row is 4-20KB contiguous DRAM. The device computes, per expert:
up.T = wupT-chunks.T @ xT (PSUM, fp32 accumulate), SwiGLU in the transposed
layout (no on-chip transposes anywhere), down = actT.T @ wdnT with the
routing weight applied on the PSUM->SBUF copy, and writes y [2*CAP, K]. The
host scatter-adds y rows back to tokens (the "combine").

Loads all go on the sync-engine HWDGE queue in consumption order (a single
queue fans out across all 16 SDMA engines and sustains full bandwidth; every
alternative — parallel queues, reorders, SWDGE offload — measured slower).
Stores alternate the sync/scalar queues so they never sit behind loads. The
leading wait-free triggers are hoisted into the NEFF entry block, scale ops
interleave between PSUM groups in the down phase, and Tile's redundant exit
semaphore-clear/barrier is stripped.
"""

import os

import ml_dtypes
import numpy as np

import concourse.bass as bass
import concourse.mybir as mybir
from concourse.bass_utils import run_bass_kernel_spmd
from concourse.tile import TileContext

T, K, H, E, TOPK = 4096, 1024, 256, 16, 2
H2 = 2 * H  # 512
NCORES = 8
EPC = E // NCORES  # experts per core = 2
CAP = 640  # token-pair capacity per expert (max observed ~550 of mean 512)
PAIRS = EPC * CAP  # 1280 rows per core
UPCHUNK = 320  # up-GEMM token tile (>=256 keeps fp32r at full rate)
KC = K // 128  # 8 contraction chunks
NT = CAP // UPCHUNK  # up token-tiles per expert
ND = CAP // 128  # down token-tiles per expert

F32 = mybir.dt.float32
# matmul input dtype: "f32r" (tf32-like, full DMA bytes) or "bf16" (half DMA)
MM_DTYPE = os.environ.get("MOE_MM_DTYPE", "f32r")


def _fix_multi_waits(nc):
    """This walrus build accepts one sync-wait command per instruction (two
    for EventSemaphore); Tile's exit drain stacks every outstanding semaphore
    onto a single Drain. Move the excess waits onto no-ops inserted before
    the offending instruction on the same engine."""
    for f in nc.m.functions:
        for bb in f.blocks:
            i = 0
            while i < len(bb.instructions):
                ins = bb.instructions[i]
                si = ins.sync_info
                cap = 2 if isinstance(ins, mybir.InstEventSemaphore) else 1
                if si is not None and si.on_wait and len(si.on_wait) > cap:
                    waits = list(si.on_wait)
                    keep, extra = waits[:cap], waits[cap:]
                    nops = [
                        mybir.InstNoOp(
                            name=f"{ins.name}_waitfix{j}",
                            sync_info=mybir.SyncInfo(on_wait=[w], on_update=[]),
                            bass_nofuse=True,
                            engine=ins.engine,
                        )
                        for j, w in enumerate(extra)
                    ]
                    ins.sync_info = mybir.SyncInfo(
                        on_wait=keep, on_update=list(si.on_update)
                    )
                    bb.instructions[i:i] = nops
                    i += len(nops)
                i += 1


_NC = None


def _build():
    global _NC
    if _NC is not None:
        return _NC
    DT = mybir.dt.float32r if MM_DTYPE == "f32r" else mybir.dt.bfloat16
    nc = bass.Bass()
    xT = nc.dram_tensor("xT", [EPC * 128, KC * CAP], DT, kind="ExternalInput")
    wup = nc.dram_tensor("wup", [EPC * 128, KC * H2], DT, kind="ExternalInput")
    wdn = nc.dram_tensor("wdn", [EPC * 2 * 128, K], DT, kind="ExternalInput")
    wv = nc.dram_tensor("wv", [128, PAIRS // 128], F32, kind="ExternalInput")
    y = nc.dram_tensor("y", [PAIRS, K], F32, kind="ExternalOutput")

    with TileContext(nc) as tc:
        with (
            tc.tile_pool(name="persist", bufs=1) as pp,
            tc.tile_pool(name="sil", bufs=4) as silp,
            tc.tile_pool(name="yout", bufs=6) as yp,
            tc.tile_pool(name="psum_up", bufs=2, space="PSUM") as psu,
            tc.tile_pool(name="psum_dn", bufs=2, space="PSUM") as psd,
        ):
            # one tile per (tensor, expert, kc-pair) so readers only gate on
            # the DMA that actually feeds them
            xsb = [
                [
                    pp.tile(
                        [128, 2, CAP], DT, tag=f"x{el}_{g}", name=f"x{el}_{g}"
                    )
                    for g in range(4)
                ]
                for el in range(EPC)
            ]
            wupsb = [
                [
                    pp.tile(
                        [128, 2, H2], DT, tag=f"wu{el}_{kg}", name=f"wu{el}_{kg}"
                    )
                    for kg in range(4)
                ]
                for el in range(EPC)
            ]
            wdnsb = [
                pp.tile([128, 2, K], DT, tag=f"wd{el}", name=f"wd{el}")
                for el in range(EPC)
            ]
            actsb = [
                [
                    pp.tile([128, CAP], DT, tag=f"a{el}_{hh}", name=f"a{el}_{hh}")
                    for hh in range(2)
                ]
                for el in range(EPC)
            ]
            wvsb = pp.tile([128, PAIRS // 128], F32)

            # all loads on the sync HWDGE queue, in consumption order
            def load_wup(el, kg):
                nc.sync.dma_start(
                    wupsb[el][kg][:],
                    wup[
                        el * 128 : (el + 1) * 128,
                        kg * 2 * H2 : (kg + 1) * 2 * H2,
                    ].rearrange("p (kc f) -> p kc f", kc=2),
                )

            def load_x(el, g):
                nc.sync.dma_start(
                    xsb[el][g][:],
                    xT[
                        el * 128 : (el + 1) * 128,
                        g * 2 * CAP : (g + 1) * 2 * CAP,
                    ].rearrange("p (kc j) -> p kc j", kc=2),
                )

            def load_wdn(el):
                r = el * 2 * 128
                nc.sync.dma_start(
                    wdnsb[el][:],
                    wdn[r : r + 256, :].rearrange("(hh p) k -> p hh k", p=128),
                )

            for g in range(4):
                load_wup(0, g)
                load_x(0, g)
            for g in range(4):
                load_wup(1, g)
            load_x(1, 0)
            load_x(1, 1)
            load_wdn(0)
            nc.sync.dma_start(wvsb[:], wv[:, :])
            load_x(1, 2)
            load_x(1, 3)
            load_wdn(1)

            def up_phase(el):
                # up.T in PSUM: [feature-on-partition, token-free]. Features
                # hh*128..hh*128+127 (gate) pair with 256+hh*128.. (proj);
                # process one hh-half at a time so only two PSUM tags are
                # live and halves pipeline through 2 bufs each.
                for ti in range(NT):
                    c0 = ti * UPCHUNK
                    for hh in range(2):
                        pg = psu.tile([128, 512], F32, tag="upA", name="pg")[
                            :, :UPCHUNK
                        ]
                        pj = psu.tile([128, 512], F32, tag="upB", name="pj")[
                            :, :UPCHUNK
                        ]
                        for kc in range(KC):
                            rhs = xsb[el][kc // 2][:, kc % 2, c0 : c0 + UPCHUNK]
                            nc.tensor.matmul(
                                pg,
                                wupsb[el][kc // 2][
                                    :, kc % 2, hh * 128 : (hh + 1) * 128
                                ],
                                rhs,
                                start=(kc == 0),
                                stop=(kc == KC - 1),
                            )
                            nc.tensor.matmul(
                                pj,
                                wupsb[el][kc // 2][
                                    :, kc % 2, 256 + hh * 128 : 384 + hh * 128
                                ],
                                rhs,
                                start=(kc == 0),
                                stop=(kc == KC - 1),
                            )
                        sil = silp.tile([128, UPCHUNK], F32, tag="sil")
                        nc.scalar.activation(
                            sil[:], pg, mybir.ActivationFunctionType.Silu
                        )
                        nc.vector.tensor_tensor(
                            actsb[el][hh][:, c0 : c0 + UPCHUNK],
                            sil[:],
                            pj,
                            mybir.AluOpType.mult,
                        )

            def down_phase(el):
                # down: [token-on-partition, k-free]; routing weight applied
                # on the PSUM->SBUF copy (split across DVE and ACT); stores
                # batch 2 token-chunks per DMA and go on the scalar/gpsimd
                # queues so they never sit behind loads on the sync queue
                for td in range(ND):
                    ysb = yp.tile([128, K], F32, tag="y", name="ysb")
                    col = el * ND + td
                    wcol = wvsb[:, col : col + 1]
                    pys = [
                        psd.tile([128, 512], F32, tag=f"dn{nn}", name=f"dn{nn}")
                        for nn in range(2)
                    ]
                    # scale of the first half runs while the second half's
                    # matmuls stream, shortening the per-block PSUM recycle
                    # and the end-of-kernel chain
                    for nn in range(2):
                        for hh in range(2):
                            nc.tensor.matmul(
                                pys[nn][:],
                                actsb[el][hh][:, td * 128 : (td + 1) * 128],
                                wdnsb[el][:, hh, nn * 512 : (nn + 1) * 512],
                                start=(hh == 0),
                                stop=(hh == 1),
                            )
                        if nn == 0:
                            nc.vector.tensor_scalar_mul(
                                ysb[:, 0:512], pys[0][:], wcol
                            )
                    nc.scalar.mul(ysb[:, 512:1024], pys[1][:], wcol)
                    r0 = el * CAP + td * 128
                    eng = nc.sync if (el * ND + td) % 2 == 0 else nc.scalar
                    eng.dma_start(y[r0 : r0 + 128, :], ysb[:])

            up_phase(0)
            up_phase(1)
            down_phase(0)
            down_phase(1)

    # Hoist the leading wait-free sync-engine DMA triggers (expert-0's
    # working set) into the entry block, ahead of the Tile entry barrier:
    # the transfers then stream during the ~6us preamble (IRAM loads, const
    # memsets, barrier) instead of after it. HWDGE triggers retire at
    # descriptor push, so the preamble barrier's Drain does not stall on
    # the in-flight transfers; the body's existing semaphore waits gate
    # consumers exactly as before.
    f0 = nc.m.functions[0]
    blocks = list(f0.blocks)
    main_bb, body_bb = blocks[0], blocks[1]
    hoist = []
    for ins in body_bb.instructions:
        if (
            isinstance(ins, mybir.InstDMACopy)
            and str(ins.engine) == "EngineType.SP"
            and not (ins.sync_info and ins.sync_info.on_wait)
        ):
            hoist.append(ins)
            if len(hoist) >= 8:
                break
        elif isinstance(ins, mybir.InstDMACopy) and str(ins.engine) == (
            "EngineType.SP"
        ):
            break
    if hoist:
        names = {h.name for h in hoist}
        body_bb.instructions[:] = [
            i for i in body_bb.instructions if i.name not in names
        ]
        ip = 0
        for idx, ins in enumerate(main_bb.instructions):
            if str(ins.engine) == "EngineType.SP":
                si = ins.sync_info
                if si and (si.on_wait or si.on_update):
                    break
                ip = idx + 1
        main_bb.instructions[ip:ip] = hoist

    if True:  # drop Tile's exit sem-clear + second barrier (redundant with
        # the compiler's own per-engine semaphore-reset epilogue; verified
        # correct across repeated executions of the loaded NEFF)
        f = nc.m.functions[0]
        endbb = list(f.blocks)[-1]
        # keep: waitfix nops + SP drain + barrier #1 (ends at the Pool
        # release EventSemaphore); drop: sem range-clear + barrier #2
        keep = []
        barrier_done = 0
        for ins in endbb.instructions:
            if barrier_done >= 1 and isinstance(
                ins, (mybir.InstDrain, mybir.InstISA)
            ):
                continue
            if barrier_done >= 1 and isinstance(ins, mybir.InstEventSemaphore):
                continue
            keep.append(ins)
            si = ins.sync_info
            if (
                isinstance(ins, mybir.InstEventSemaphore)
                and si
                and si.on_update
                and si.on_update[0].update_mode == "sem-add-imm"
                and si.on_update[0].update_value == 4
            ):
                barrier_done += 1
        endbb.instructions[:] = keep
    _fix_multi_waits(nc)
    _NC = nc
    return nc


last_results = None  # BassKernelResults of the most recent launch (for test.py)


def _pack_pkc(a, inner):
    """[KC*128, inner] -> [128, KC*inner] with row p holding [kc, inner]."""
    return (
        a.reshape(KC, 128, inner).transpose(1, 0, 2).reshape(128, KC * inner)
    )


def kernel(hidden_states, topk_weights, topk_ids, up_weight, down_weight):
    global last_results
    np_dt = np.float32 if MM_DTYPE == "f32r" else ml_dtypes.bfloat16
    hs = np.asarray(hidden_states, dtype=np.float32)
    twf = np.asarray(topk_weights, dtype=np.float32).ravel()
    ids = np.asarray(topk_ids).astype(np.int64).ravel()
    wu = np.asarray(up_weight, dtype=np.float32)
    wd = np.asarray(down_weight, dtype=np.float32)

    nc = _build()

    order = np.argsort(ids, kind="stable")
    counts = np.bincount(ids, minlength=E)
    starts = np.concatenate([[0], np.cumsum(counts)])
    hsT = np.ascontiguousarray(hs.T.astype(np_dt))  # [K, T]

    wup_maps = []
    wdn_maps = []
    for c in range(NCORES):
        es = range(EPC * c, EPC * (c + 1))
        wup_maps.append(
            np.ascontiguousarray(
                np.stack([_pack_pkc(wu[e].T.astype(np_dt), H2) for e in es])
            ).reshape(EPC * 128, KC * H2)
        )
        wdn_maps.append(
            np.ascontiguousarray(
                np.concatenate([wd[e].T.astype(np_dt) for e in es], axis=0)
            )
        )

    out = np.zeros((T, K), np.float32)
    rounds = int(max(1, -(-int(counts.max()) // CAP)))
    for r in range(rounds):
        in_maps = []
        toks = []  # per core: list of (el, n, token_idx)
        for c in range(NCORES):
            xTa = np.zeros((EPC, 128, KC, CAP), np_dt)
            wva = np.zeros((PAIRS // 128, 128), np.float32)
            ct = []
            for el in range(EPC):
                e = EPC * c + el
                lo = starts[e] + r * CAP
                hi = min(starts[e + 1], lo + CAP)
                seg = order[lo:hi] if hi > lo else np.empty(0, np.int64)
                n = len(seg)
                if n:
                    t = seg // TOPK
                    g = hsT[:, t].reshape(KC, 128, n)  # [kc, p, n]
                    xTa[el, :, :, :n] = g.transpose(1, 0, 2)
                    wva.reshape(-1)[el * CAP : el * CAP + n] = twf[seg]
                    ct.append((el, n, t))
            toks.append(ct)
            in_maps.append(
                {
                    "xT": xTa.reshape(EPC * 128, KC * CAP),
                    "wup": wup_maps[c],
                    "wdn": wdn_maps[c],
                    "wv": np.ascontiguousarray(wva.T),
                }
            )
        last_results = run_bass_kernel_spmd(
            nc, in_maps, core_ids=list(range(NCORES))
        )
        for c in range(NCORES):
            yc = last_results.results[c]["y"]
            for el, n, t in toks[c]:
                np.add.at(out, t, yc[el * CAP : el * CAP + n])
    return out



# revision 2
# speedup vs baseline: 1.3138x; 1.3138x over previous
"""Fused MoE (top-2 routing) on 8 trn2 NeuronCores, expert-parallel.

Strategy: E=16 experts are sharded 2-per-core. The host groups the T*TOPK
(token, slot) pairs by expert (the all-to-all "dispatch"), pads each expert's
token list to a fixed capacity CAP, and ships each core pre-transposed,
SBUF-layout-matched blocks:
  - xT  [2*128, 8*CAP]   gathered tokens: row el*128+p, col kc*CAP+j holds
                         x[token j of expert el, k=kc*128+p]
  - wup [2*128, 8*512]   up_weight[e].T in the same [p, kc, f] layout
  - wdn [4*128, 1024]    down_weight[e].T, row (el*2+hh)*128+p, col kout
  - wv  [128, 2*ND]      routing weight per pair, [p, tile] layout
Every DMA row is 2-9KB contiguous DRAM. All matmul IO is fp16 (same 10-bit
mantissa as the tf32 path it replaced, half the HBM bytes; PSUM accumulates
fp32). The device computes, per expert: up.T = wupT-chunks.T @ xT (PSUM,
fp32 accumulate), SwiGLU in the transposed layout (no on-chip transposes
anywhere), down = actT.T @ wdnT with the routing weight applied on the
PSUM->SBUF copy, and writes y [2*CAP, K] fp16. The host scatter-adds y rows
back to tokens (the "combine").

Loads all go on the sync-engine HWDGE queue in consumption order (a single
queue fans out across all 16 SDMA engines and sustains full bandwidth; every
alternative — parallel queues, reorders, SWDGE offload — measured slower).
Stores alternate the sync/scalar queues so they never sit behind loads. The
leading wait-free triggers are hoisted into the NEFF entry block, scale ops
interleave between PSUM groups in the down phase, and Tile's redundant exit
semaphore-clear/barrier is stripped.
"""

import os

import numpy as np

import concourse.bass as bass
import concourse.mybir as mybir
from concourse.bass_utils import run_bass_kernel_spmd
from concourse.tile import TileContext

T, K, H, E, TOPK = 4096, 1024, 256, 16, 2
H2 = 2 * H  # 512
NCORES = 8
EPC = E // NCORES  # experts per core = 2
CAP = 576  # token-pair capacity per expert (max observed 550 of mean 512)
PAIRS = EPC * CAP  # 1152 rows per core
UPCHUNK = 288  # up-GEMM token tile
KC = K // 128  # 8 contraction chunks
NT = CAP // UPCHUNK  # up token-tiles per expert
ND = -(-CAP // 128)  # down token-tiles per expert (last one partial)
DTAIL = CAP - (ND - 1) * 128  # tokens in the last down tile

F32 = mybir.dt.float32
DT = mybir.dt.float16
NP_DT = np.float16


def _fix_multi_waits(nc):
    """This walrus build accepts one sync-wait command per instruction (two
    for EventSemaphore); Tile's exit drain stacks every outstanding semaphore
    onto a single Drain. Move the excess waits onto no-ops inserted before
    the offending instruction on the same engine."""
    for f in nc.m.functions:
        for bb in f.blocks:
            i = 0
            while i < len(bb.instructions):
                ins = bb.instructions[i]
                si = ins.sync_info
                cap = 2 if isinstance(ins, mybir.InstEventSemaphore) else 1
                if si is not None and si.on_wait and len(si.on_wait) > cap:
                    waits = list(si.on_wait)
                    keep, extra = waits[:cap], waits[cap:]
                    nops = [
                        mybir.InstNoOp(
                            name=f"{ins.name}_waitfix{j}",
                            sync_info=mybir.SyncInfo(on_wait=[w], on_update=[]),
                            bass_nofuse=True,
                            engine=ins.engine,
                        )
                        for j, w in enumerate(extra)
                    ]
                    ins.sync_info = mybir.SyncInfo(
                        on_wait=keep, on_update=list(si.on_update)
                    )
                    bb.instructions[i:i] = nops
                    i += len(nops)
                i += 1


_NC = None


def _build():
    global _NC
    if _NC is not None:
        return _NC
    nc = bass.Bass()
    xT = nc.dram_tensor("xT", [EPC * 128, KC * CAP], DT, kind="ExternalInput")
    wup = nc.dram_tensor("wup", [EPC * 128, KC * H2], DT, kind="ExternalInput")
    wdn = nc.dram_tensor("wdn", [EPC * 2 * 128, K], DT, kind="ExternalInput")
    wv = nc.dram_tensor("wv", [128, EPC * ND], F32, kind="ExternalInput")
    y = nc.dram_tensor("y", [PAIRS, K], DT, kind="ExternalOutput")

    with TileContext(nc) as tc:
        with (
            tc.tile_pool(name="persist", bufs=1) as pp,
            tc.tile_pool(name="sil", bufs=4) as silp,
            tc.tile_pool(name="yout", bufs=6) as yp,
            tc.tile_pool(name="psum_up", bufs=2, space="PSUM") as psu,
            tc.tile_pool(name="psum_dn", bufs=2, space="PSUM") as psd,
        ):
            # one tile per (tensor, expert, kc-pair) so readers only gate on
            # the DMA that actually feeds them
            xsb = [
                [
                    pp.tile(
                        [128, 2, CAP], DT, tag=f"x{el}_{g}", name=f"x{el}_{g}"
                    )
                    for g in range(4)
                ]
                for el in range(EPC)
            ]
            wupsb = [
                [
                    pp.tile(
                        [128, 2, H2], DT, tag=f"wu{el}_{kg}", name=f"wu{el}_{kg}"
                    )
                    for kg in range(4)
                ]
                for el in range(EPC)
            ]
            wdnsb = [
                pp.tile([128, 2, K], DT, tag=f"wd{el}", name=f"wd{el}")
                for el in range(EPC)
            ]
            actsb = [
                [
                    pp.tile([128, CAP], DT, tag=f"a{el}_{hh}", name=f"a{el}_{hh}")
                    for hh in range(2)
                ]
                for el in range(EPC)
            ]
            wvsb = pp.tile([128, EPC * ND], F32)

            # all loads on the sync HWDGE queue, in consumption order
            def load_wup(el, kg):
                nc.sync.dma_start(
                    wupsb[el][kg][:],
                    wup[
                        el * 128 : (el + 1) * 128,
                        kg * 2 * H2 : (kg + 1) * 2 * H2,
                    ].rearrange("p (kc f) -> p kc f", kc=2),
                )

            def load_x(el, g):
                nc.sync.dma_start(
                    xsb[el][g][:],
                    xT[
                        el * 128 : (el + 1) * 128,
                        g * 2 * CAP : (g + 1) * 2 * CAP,
                    ].rearrange("p (kc j) -> p kc j", kc=2),
                )

            def load_wdn(el):
                r = el * 2 * 128
                nc.sync.dma_start(
                    wdnsb[el][:],
                    wdn[r : r + 256, :].rearrange("(hh p) k -> p hh k", p=128),
                )

            for g in range(4):
                load_wup(0, g)
                load_x(0, g)
            for g in range(4):
                load_wup(1, g)
            load_x(1, 0)
            load_x(1, 1)
            load_wdn(0)
            nc.sync.dma_start(wvsb[:], wv[:, :])
            load_x(1, 2)
            load_x(1, 3)
            load_wdn(1)

            def up_phase(el):
                # up.T in PSUM: [feature-on-partition, token-free]. Features
                # hh*128..hh*128+127 (gate) pair with 256+hh*128.. (proj);
                # process one hh-half at a time so only two PSUM tags are
                # live and halves pipeline through 2 bufs each.
                for ti in range(NT):
                    c0 = ti * UPCHUNK
                    for hh in range(2):
                        pg = psu.tile([128, 512], F32, tag="upA", name="pg")[
                            :, :UPCHUNK
                        ]
                        pj = psu.tile([128, 512], F32, tag="upB", name="pj")[
                            :, :UPCHUNK
                        ]
                        for kc in range(KC):
                            rhs = xsb[el][kc // 2][:, kc % 2, c0 : c0 + UPCHUNK]
                            nc.tensor.matmul(
                                pg,
                                wupsb[el][kc // 2][
                                    :, kc % 2, hh * 128 : (hh + 1) * 128
                                ],
                                rhs,
                                start=(kc == 0),
                                stop=(kc == KC - 1),
                            )
                            nc.tensor.matmul(
                                pj,
                                wupsb[el][kc // 2][
                                    :, kc % 2, 256 + hh * 128 : 384 + hh * 128
                                ],
                                rhs,
                                start=(kc == 0),
                                stop=(kc == KC - 1),
                            )
                        sil = silp.tile([128, UPCHUNK], F32, tag="sil")
                        nc.scalar.activation(
                            sil[:], pg, mybir.ActivationFunctionType.Silu
                        )
                        nc.vector.tensor_tensor(
                            actsb[el][hh][:, c0 : c0 + UPCHUNK],
                            sil[:],
                            pj,
                            mybir.AluOpType.mult,
                        )

            def down_phase(el):
                # down: [token-on-partition, k-free]; routing weight applied
                # on the PSUM->SBUF copy (split across DVE and ACT); stores
                # go on alternating sync/scalar queues so they never sit
                # behind loads. The last token-tile is partial (DTAIL rows).
                for td in range(ND):
                    nrow = 128 if td < ND - 1 else DTAIL
                    ysb = yp.tile([128, K], DT, tag="y", name="ysb")
                    col = el * ND + td
                    wcol = wvsb[:nrow, col : col + 1]
                    pys = [
                        psd.tile([128, 512], F32, tag=f"dn{nn}", name=f"dn{nn}")
                        for nn in range(2)
                    ]
                    # scale of the first half runs while the second half's
                    # matmuls stream, shortening the per-block PSUM recycle
                    # and the end-of-kernel chain
                    for nn in range(2):
                        for hh in range(2):
                            nc.tensor.matmul(
                                pys[nn][:nrow],
                                actsb[el][hh][
                                    :, td * 128 : td * 128 + nrow
                                ],
                                wdnsb[el][:, hh, nn * 512 : (nn + 1) * 512],
                                start=(hh == 0),
                                stop=(hh == 1),
                            )
                        if nn == 0:
                            nc.vector.tensor_scalar_mul(
                                ysb[:nrow, 0:512], pys[0][:nrow], wcol
                            )
                    nc.scalar.mul(ysb[:nrow, 512:1024], pys[1][:nrow], wcol)
                    r0 = el * CAP + td * 128
                    eng = nc.sync if (el * ND + td) % 2 == 0 else nc.scalar
                    eng.dma_start(y[r0 : r0 + nrow, :], ysb[:nrow])

            up_phase(0)
            up_phase(1)
            down_phase(0)
            down_phase(1)

    # Hoist the leading wait-free sync-engine DMA triggers (expert-0's
    # working set) into the entry block, ahead of the Tile entry barrier:
    # the transfers then stream during the ~6us preamble (IRAM loads, const
    # memsets, barrier) instead of after it. HWDGE triggers retire at
    # descriptor push, so the preamble barrier's Drain does not stall on
    # the in-flight transfers; the body's existing semaphore waits gate
    # consumers exactly as before.
    f0 = nc.m.functions[0]
    blocks = list(f0.blocks)
    main_bb, body_bb = blocks[0], blocks[1]
    hoist = []
    for ins in body_bb.instructions:
        if (
            isinstance(ins, mybir.InstDMACopy)
            and str(ins.engine) == "EngineType.SP"
            and not (ins.sync_info and ins.sync_info.on_wait)
        ):
            hoist.append(ins)
            if len(hoist) >= 8:
                break
        elif isinstance(ins, mybir.InstDMACopy) and str(ins.engine) == (
            "EngineType.SP"
        ):
            break
    if hoist:
        names = {h.name for h in hoist}
        body_bb.instructions[:] = [
            i for i in body_bb.instructions if i.name not in names
        ]
        ip = 0
        for idx, ins in enumerate(main_bb.instructions):
            if str(ins.engine) == "EngineType.SP":
                si = ins.sync_info
                if si and (si.on_wait or si.on_update):
                    break
                ip = idx + 1
        main_bb.instructions[ip:ip] = hoist

    if True:  # drop Tile's exit sem-clear + second barrier (redundant with
        # the compiler's own per-engine semaphore-reset epilogue; verified
        # correct across repeated executions of the loaded NEFF)
        f = nc.m.functions[0]
        endbb = list(f.blocks)[-1]
        # keep: waitfix nops + SP drain + barrier #1 (ends at the Pool
        # release EventSemaphore); drop: sem range-clear + barrier #2
        keep = []
        barrier_done = 0
        for ins in endbb.instructions:
            if barrier_done >= 1 and isinstance(
                ins, (mybir.InstDrain, mybir.InstISA)
            ):
                continue
            if barrier_done >= 1 and isinstance(ins, mybir.InstEventSemaphore):
                continue
            keep.append(ins)
            si = ins.sync_info
            if (
                isinstance(ins, mybir.InstEventSemaphore)
                and si
                and si.on_update
                and si.on_update[0].update_mode == "sem-add-imm"
                and si.on_update[0].update_value == 4
            ):
                barrier_done += 1
        endbb.instructions[:] = keep
    _fix_multi_waits(nc)
    _NC = nc
    return nc


last_results = None  # BassKernelResults of the most recent launch (for test.py)


def _pack_pkc(a, inner):
    """[KC*128, inner] -> [128, KC*inner] with row p holding [kc, inner]."""
    return (
        a.reshape(KC, 128, inner).transpose(1, 0, 2).reshape(128, KC * inner)
    )


def kernel(hidden_states, topk_weights, topk_ids, up_weight, down_weight):
    global last_results
    hs = np.asarray(hidden_states, dtype=np.float32)
    twf = np.asarray(topk_weights, dtype=np.float32).ravel()
    ids = np.asarray(topk_ids).astype(np.int64).ravel()
    wu = np.asarray(up_weight, dtype=np.float32)
    wd = np.asarray(down_weight, dtype=np.float32)

    nc = _build()

    order = np.argsort(ids, kind="stable")
    counts = np.bincount(ids, minlength=E)
    starts = np.concatenate([[0], np.cumsum(counts)])
    hsT = np.ascontiguousarray(hs.T.astype(NP_DT))  # [K, T]

    wup_maps = []
    wdn_maps = []
    for c in range(NCORES):
        es = range(EPC * c, EPC * (c + 1))
        wup_maps.append(
            np.ascontiguousarray(
                np.stack([_pack_pkc(wu[e].T.astype(NP_DT), H2) for e in es])
            ).reshape(EPC * 128, KC * H2)
        )
        wdn_maps.append(
            np.ascontiguousarray(
                np.concatenate([wd[e].T.astype(NP_DT) for e in es], axis=0)
            )
        )

    out = np.zeros((T, K), np.float32)
    rounds = int(max(1, -(-int(counts.max()) // CAP)))
    for r in range(rounds):
        in_maps = []
        toks = []  # per core: list of (el, n, token_idx)
        for c in range(NCORES):
            xTa = np.zeros((EPC, 128, KC, CAP), NP_DT)
            wva = np.zeros((EPC * ND * 128,), np.float32)
            ct = []
            for el in range(EPC):
                e = EPC * c + el
                lo = starts[e] + r * CAP
                hi = min(starts[e + 1], lo + CAP)
                seg = order[lo:hi] if hi > lo else np.empty(0, np.int64)
                n = len(seg)
                if n:
                    t = seg // TOPK
                    g = hsT[:, t].reshape(KC, 128, n)  # [kc, p, n]
                    xTa[el, :, :, :n] = g.transpose(1, 0, 2)
                    wva[el * ND * 128 : el * ND * 128 + n] = twf[seg]
                    ct.append((el, n, t))
            toks.append(ct)
            in_maps.append(
                {
                    "xT": xTa.reshape(EPC * 128, KC * CAP),
                    "wup": wup_maps[c],
                    "wdn": wdn_maps[c],
                    "wv": np.ascontiguousarray(
                        wva.reshape(EPC * ND, 128).T
                    ),
                }
            )
        last_results = run_bass_kernel_spmd(
            nc, in_maps, core_ids=list(range(NCORES))
        )
        for c in range(NCORES):
            yc = last_results.results[c]["y"].astype(np.float32)
            for el, n, t in toks[c]:
                np.add.at(out, t, yc[el * CAP : el * CAP + n])
    return out
